# revision 7
# baseline (speedup 1.0000x reference)
"""BiLSTM Enc-Dec + CRF NLL loss on ONE Trainium2 core (zero collectives).

Design (from microbenchmarking this hardware):
- Small matmuls with register-offset (loop-var) access patterns cost ~300ns
  each; with constant offsets ~15ns. So the recurrent h@W_hh matvec keeps h
  in a (U+1)-slot ping-pong buffer indexed by the Python-unrolled step index:
  all 64 PE matmuls per step use constant APs. The time-indexed history write
  (needed by the next layer's input projection) is a batched DVE copy per
  unrolled body (register offsets are cheap on DVE).
- Collectives cost ~41 ms fixed per execution on this stack, so everything
  runs on core 0: the fwd and bwd direction scans of each layer are
  interleaved step-by-step on one core, which hides most of each scan's
  serial chain latency in the other's engine gaps.
- The per-step xp (input projection) add is folded into PSUM via an identity
  matmul (start=True) so activations read gates straight from PSUM.
- CRF partition function: linear domain with renorm every 8 steps; split
  into a forward alpha recursion over t=0..1023 and a backward beta
  recursion over t=2047..1024, interleaved on the same engines; the host
  sums the logs of the stored norms in float64.
"""

import sys

sys.path.insert(0, "/opt/trn_rl_repo")

import numpy as np
import ml_dtypes

import concourse.bacc as bacc
import concourse.mybir as mybir
from concourse.bass import ds
from concourse.tile import TileContext
from concourse.bass_utils import run_bass_kernel_spmd

T = 2048
ELMO = 1024
H = 512
POS = 64
K = 48
S = 50
L = 2
NEG = -10000.0
START_IDX, END_IDX = 0, 1

Din0 = ELMO + POS  # 1088
K0C = 9  # ceil(1088/128)
HC = 4
G = 4 * H  # 2048
GC = 16
U = 8  # unrolled steps per hardware-loop body
CH = 128  # steps per xp window
NT = 512  # time-block for bulk matmuls
RN = 8  # CRF renorm cadence
TH = T // 2  # alpha/beta split point

bf16 = mybir.dt.bfloat16
f32 = mybir.dt.float32
AF = mybir.ActivationFunctionType
ALU = mybir.AluOpType

_CACHE = {}

STAGES = [("enc", 0), ("enc", 1), ("dec", 0), ("dec", 1)]

# all big bf16 inputs are packed into ONE [128, MEGA_COLS] tensor: several
# multi-MB separate input tensors cost ~10 ms EACH per execution on this
# stack, while one giant tensor (or many tiny ones) is free.
MEGA = [("embT", K0C * T), ("ident", 128)]
for _m in ("enc", "dec"):
    for _l in (0, 1):
        for _d in (0, 1):
            MEGA.append((f"whhT_{_m}{_l}_{_d}", HC * G))
    for _d in (0, 1):
        MEGA.append((f"wih0T_{_m}_{_d}", K0C * G))
        for _s in ("f", "b"):
            MEGA.append((f"wih1T_{_m}_{_d}_{_s}", HC * G))
MEGA += [("e2hT", GC * G), ("e2cT", GC * G), ("h2tT_f", HC * K), ("h2tT_b", HC * K)]
MEGA_OFF = {}
_o = 0
for _n, _w in MEGA:
    MEGA_OFF[_n] = _o
    _o += _w
MEGA_COLS = _o


# ----------------------------------------------------------------------------
# host-side weight preparation
# ----------------------------------------------------------------------------

def _perm_gates(a):
    """reorder gate rows [i,f,g,o] -> [i,f,o,g] along axis 0 (size 4H)."""
    return np.concatenate([a[0:H], a[H : 2 * H], a[3 * H : 4 * H], a[2 * H : 3 * H]], 0)


def _tile_kT(wT, nk):
    """[Ktot, M] -> [128, nk*M] with col kc*M + m = wT[kc*128 + p, m]."""
    Ktot, M = wT.shape
    assert Ktot == nk * 128
    return np.ascontiguousarray(wT.reshape(nk, 128, M).transpose(1, 0, 2).reshape(128, nk * M))


def _prep(inputs):
    f = np.float32
    ins = {}
    sentence = inputs["sentence"].astype(f)
    pos_emb = inputs["pos_emb"].astype(f)
    speech = np.asarray(inputs["speech_tags"]).astype(np.int64)
    embeds = np.concatenate([sentence, pos_emb[speech]], axis=1)  # (T, 1088)
    embT = np.zeros((K0C * 128, T), f)
    embT[:Din0] = embeds.T
    ins["embT"] = _tile_kT(embT, K0C).astype(ml_dtypes.bfloat16)
    ins["ident"] = np.eye(128).astype(ml_dtypes.bfloat16)

    for m in ("enc", "dec"):
        for l in (0, 1):
            for d in (0, 1):
                whh = _perm_gates(inputs[f"{m}_w_hh{l}"][d].astype(f))
                ins[f"whhT_{m}{l}_{d}"] = _tile_kT(
                    np.ascontiguousarray(whh.T), HC
                ).astype(ml_dtypes.bfloat16)
                b = _perm_gates(
                    (inputs[f"{m}_b_ih{l}"][d] + inputs[f"{m}_b_hh{l}"][d]).astype(f)
                )
                ins[f"bias_{m}{l}_{d}"] = np.ascontiguousarray(
                    b.reshape(GC, 128).T
                ).astype(f)
        for d in (0, 1):
            wih0 = _perm_gates(inputs[f"{m}_w_ih0"][d].astype(f))  # (2048, 1088)
            w0T = np.zeros((K0C * 128, G), f)
            w0T[:Din0] = wih0.T
            ins[f"wih0T_{m}_{d}"] = _tile_kT(w0T, K0C).astype(ml_dtypes.bfloat16)
            wih1 = _perm_gates(inputs[f"{m}_w_ih1"][d].astype(f))  # (2048, 1024)
            wf = wih1[:, 0:H]  # multiplies fwd-dir L0 outputs
            wb = wih1[:, H : 2 * H]  # multiplies bwd-dir L0 outputs
            ins[f"wih1T_{m}_{d}_f"] = _tile_kT(
                np.ascontiguousarray(wf.T), HC
            ).astype(ml_dtypes.bfloat16)
            ins[f"wih1T_{m}_{d}_b"] = _tile_kT(
                np.ascontiguousarray(wb.T), HC
            ).astype(ml_dtypes.bfloat16)

    # e2h/e2c: natural order both sides. out rows = [dl0f dl0b dl1f dl1b],
    # in cols = [el0f el0b el1f el1b] (PyTorch flat order of (2L, H) states).
    for nm in ("e2h", "e2c"):
        w = inputs[f"{nm}_w"].astype(f)  # (2048, 2048)
        ins[f"{nm}T"] = _tile_kT(np.ascontiguousarray(w.T), GC).astype(ml_dtypes.bfloat16)
        ins[f"{nm}_b"] = np.ascontiguousarray(
            inputs[f"{nm}_b"].astype(f).reshape(GC, 128).T
        ).astype(f)

    h2t = inputs["h2t_w"].astype(f)  # (K, 1024)
    ins["h2tT_f"] = _tile_kT(np.ascontiguousarray(h2t[:, 0:H].T), HC).astype(
        ml_dtypes.bfloat16
    )
    ins["h2tT_b"] = _tile_kT(np.ascontiguousarray(h2t[:, H:].T), HC).astype(
        ml_dtypes.bfloat16
    )
    ins["h2t_b"] = inputs["h2t_b"].astype(f).reshape(K, 1)

    trans = inputs["transitions"].astype(np.float64)
    E = np.exp(trans).astype(f)  # E[next, prev]
    ins["EexpT"] = np.ascontiguousarray(E.T)  # lhsT for alpha: out = E @ x
    ins["Eexp"] = np.ascontiguousarray(E)  # lhsT for beta: out = E^T @ x
    ins["betaT"] = np.ascontiguousarray(E[END_IDX].reshape(K, 1))  # exp(trans[END])
    a0 = np.zeros((K, 1), f)
    a0[START_IDX, 0] = 1.0
    ins["alpha0"] = a0
    ins["ones48"] = np.ones((K, K), f)
    mega = np.empty((128, MEGA_COLS), ml_dtypes.bfloat16)
    for n, w in MEGA:
        mega[:, MEGA_OFF[n] : MEGA_OFF[n] + w] = ins.pop(n)
    ins["mega"] = mega
    return ins


# ----------------------------------------------------------------------------
# device program
# ----------------------------------------------------------------------------

def build():
    import os
    skips = set(os.environ.get("BK_SKIP", "").split(","))
    nc = bacc.Bacc("TRN2", target_bir_lowering=False, num_devices=1)

    def din(name, shape, dt=bf16):
        return nc.dram_tensor(name, shape, dt, kind="ExternalInput")

    mega_d = din("mega", [128, MEGA_COLS])

    def mega_ap(name):
        o = MEGA_OFF[name]
        return mega_d[:, o : o + dict(MEGA)[name]]

    embT_ap = mega_ap("embT")
    ident_ap = mega_ap("ident")
    whh_ap = {
        (m, l, d): mega_ap(f"whhT_{m}{l}_{d}")
        for m in ("enc", "dec") for l in (0, 1) for d in (0, 1)
    }
    bias_d = {
        (m, l, d): din(f"bias_{m}{l}_{d}", [128, GC], f32)
        for m in ("enc", "dec") for l in (0, 1) for d in (0, 1)
    }
    wih0_ap = {(m, d): mega_ap(f"wih0T_{m}_{d}") for m in ("enc", "dec") for d in (0, 1)}
    wih1_ap = {
        (m, d, s): mega_ap(f"wih1T_{m}_{d}_{s}")
        for m in ("enc", "dec") for d in (0, 1) for s in ("f", "b")
    }
    e2hT_ap = mega_ap("e2hT")
    e2cT_ap = mega_ap("e2cT")
    e2hb_d = din("e2h_b", [128, GC], f32)
    e2cb_d = din("e2c_b", [128, GC], f32)
    h2tT_f_ap = mega_ap("h2tT_f")
    h2tT_b_ap = mega_ap("h2tT_b")
    h2tb_d = din("h2t_b", [K, 1], f32)
    EexpT_d = din("EexpT", [K, K], f32)
    Eexp_d = din("Eexp", [K, K], f32)
    betaT_d = din("betaT", [K, 1], f32)
    alpha0_d = din("alpha0", [K, 1], f32)
    ones48_d = din("ones48", [K, K], f32)

    feats_out = nc.dram_tensor("feats", [K, T], f32, kind="ExternalOutput")
    NSA = TH // RN  # 128 alpha norms
    NSB = (T - TH) // RN  # 128 beta norms
    snorm_out = nc.dram_tensor("snorm", [1, NSA + NSB + 1], f32, kind="ExternalOutput")

    # internal DRAM xp buffers: layer 1 reuses layer 0's (its xp is fully
    # consumed by the L0 scan before xp_l1 rewrites the buffer)
    xp_dram = {}
    for m in ("enc", "dec"):
        for d in (0, 1):
            buf = nc.dram_tensor(f"xp_{m}_{d}", [128, GC * T], bf16)
            xp_dram[(m, 0, d)] = buf
            xp_dram[(m, 1, d)] = buf

    with TileContext(nc) as tc:
        with (
            tc.tile_pool(name="pw", bufs=1) as pw,
            tc.tile_pool(name="wslab", bufs=2) as wslab_pool,  # whh / wih1 slabs
            tc.tile_pool(name="hs", bufs=3) as hs_pool,
            tc.tile_pool(name="win", bufs=2) as win_pool,  # streamed emb windows
            tc.tile_pool(name="w0s", bufs=2) as w0s_pool,  # wih0 half-slab windows
            tc.tile_pool(name="xw", bufs=2) as xw_pool,  # xp scan windows
            tc.tile_pool(name="xst", bufs=6) as xst_pool,  # xp store staging
            tc.tile_pool(name="psx", bufs=2, space="PSUM") as psx_pool,
            tc.tile_pool(name="pss", bufs=3, space="PSUM") as pss_pool,
        ):
            ident = pw.tile([128, 128], bf16, name="ident")
            nc.sync.dma_start(out=ident, in_=ident_ap)

            # ================= P0: layer-0 xp for all 4 (model, dir) =========
            embr = embT_ap.rearrange("p (k t) -> p k t", k=K0C)
            for m in ("enc", "dec") if "p0" not in skips else ():
                for d in (0, 1):
                    bias = pw.tile([128, GC], f32, tag="bias0", name=f"b0_{m}{d}")
                    nc.sync.dma_start(out=bias, in_=bias_d[(m, 0, d)][:, :])
                    w0r = wih0_ap[(m, d)].rearrange("p (k g) -> p k g", k=K0C)
                    w0h = []
                    for half in (0, 1):
                        w0t = w0s_pool.tile(
                            [128, K0C, G // 2], bf16, tag="w0h", name=f"w0_{m}{d}_{half}"
                        )
                        nc.sync.dma_start(
                            out=w0t, in_=w0r[:, :, half * (G // 2) : (half + 1) * (G // 2)]
                        )
                        w0h.append(w0t)
                    for tb in range(T // NT):
                        t0 = tb * NT
                        ew = win_pool.tile([128, K0C, NT], bf16, tag="ew", name=f"ew_{m}{d}_{tb}")
                        nc.sync.dma_start(out=ew, in_=embr[:, :, t0 : t0 + NT])
                        if d == 0:
                            mv = ew[:, :, :]
                        else:
                            # bwd dir: reversed time; psum col j = bwd-step
                            # s = (T - t0 - NT) + j
                            mv = ew[:, :, NT - 1 :: -1]
                        s0 = t0 if d == 0 else T - t0 - NT
                        for mc in range(GC):
                            wt = w0h[mc // 8]
                            mo = (mc % 8) * 128
                            ps = psx_pool.tile([128, NT], f32, tag="psx", name=f"ps0_{m}{d}_{tb}_{mc}")
                            for kc in range(K0C):
                                nc.tensor.matmul(
                                    ps, wt[:, kc, mo : mo + 128], mv[:, kc, :],
                                    start=(kc == 0), stop=(kc == K0C - 1),
                                )
                            st = xst_pool.tile([128, NT], bf16, tag="xst", name=f"st0_{m}{d}_{tb}_{mc}")
                            nc.vector.tensor_scalar(
                                out=st, in0=ps, scalar1=bias[:, mc : mc + 1],
                                scalar2=None, op0=ALU.add,
                            )
                            nc.sync.dma_start(
                                out=xp_dram[(m, 0, d)][:, mc * T + s0 : mc * T + s0 + NT],
                                in_=st,
                            )

            # ================= scan machinery ================================
            def pair_scan(m, l, Hs_f, Hs_b, init_h=None, init_c=None):
                """Interleaved fwd/bwd scan for stage (m, l). Hs_* are
                [128, HC*(T+1)] bf16 history tiles. init_h/init_c are
                ([128,16] bf16, [128,16] f32) tiles; columns 4*scan_idx.. hold
                the state for scan (l, d). Returns (c_f, c_b) f32 tiles."""
                dirs = []
                for d in (0, 1):
                    W = wslab_pool.tile([128, HC * G], bf16, tag="wslab", name=f"whh_{m}{l}_{d}")
                    nc.sync.dma_start(out=W, in_=whh_ap[(m, l, d)])
                    hp = pw.tile([128, U + 1, HC], bf16, tag=f"hp{d}", name=f"hp_{m}{l}_{d}")
                    cd = pw.tile([128, HC], f32, tag=f"c{d}", name=f"c_{m}{l}_{d}")
                    si = 2 * l + d
                    if init_h is None:
                        nc.vector.memset(hp[:, 0, :], 0.0)
                        nc.vector.memset(cd, 0.0)
                    else:
                        nc.vector.tensor_copy(hp[:, 0, :], init_h[:, 4 * si : 4 * si + 4])
                        nc.vector.tensor_copy(cd, init_c[:, 4 * si : 4 * si + 4])
                    sg = pw.tile([128, 12], f32, tag=f"sg{d}", name=f"sg_{m}{l}_{d}")
                    tg = pw.tile([128, 4], f32, tag=f"tg{d}", name=f"tg_{m}{l}_{d}")
                    t1 = pw.tile([128, 4], f32, tag=f"t1{d}", name=f"t1_{m}{l}_{d}")
                    t2 = pw.tile([128, 4], f32, tag=f"t2{d}", name=f"t2_{m}{l}_{d}")
                    tn = pw.tile([128, 4], f32, tag=f"tn{d}", name=f"tn_{m}{l}_{d}")
                    dirs.append([W, hp, cd, sg, tg, t1, t2, tn, None])

                xpr = {
                    d: xp_dram[(m, l, d)][:, :].rearrange("p (g t) -> p g t", g=GC)
                    for d in (0, 1)
                }
                for w in range(T // CH):
                    t0 = w * CH
                    for d in (0, 1):
                        xwt = xw_pool.tile(
                            [128, GC, CH], bf16, tag=f"xw{d}", name=f"xw_{m}{l}_{d}_{w}"
                        )
                        nc.sync.dma_start(out=xwt, in_=xpr[d][:, :, t0 : t0 + CH])
                        dirs[d][8] = xwt
                    xc = [
                        pw.tile([128, GC, U], bf16, tag=f"xc{d}", name=f"xc_{m}{l}_{d}_{w}")
                        for d in (0, 1)
                    ]
                    with tc.For_i(0, CH // U) as iv:
                        for d in (0, 1):
                            nc.vector.tensor_copy(
                                xc[d],
                                dirs[d][8][:, :, ds(U * iv, U)],
                            )
                        for u in range(U):
                            pps = []
                            for d in (0, 1):
                                W, hp = dirs[d][0], dirs[d][1]
                                ps = pss_pool.tile(
                                    [128, GC], f32, tag=f"ps{d}", name=f"ps_{m}{l}_{d}_{u}"
                                )
                                nc.tensor.matmul(
                                    ps, ident, xc[d][:, :, u], start=True, stop=False,
                                    skip_group_check=True,
                                )
                                for mc in range(GC):
                                    for kc in range(HC):
                                        nc.tensor.matmul(
                                            ps[:, mc : mc + 1],
                                            W[:, kc * G + mc * 128 : kc * G + (mc + 1) * 128],
                                            hp[:, u, kc : kc + 1],
                                            start=False,
                                            stop=(kc == HC - 1),
                                            skip_group_check=True,
                                        )
                                pps.append(ps)
                            for d in (0, 1):
                                sg, tg = dirs[d][3], dirs[d][4]
                                nc.scalar.activation(sg, pps[d][:, 0:12], AF.Sigmoid)
                                nc.scalar.activation(tg, pps[d][:, 12:16], AF.Tanh)
                            for d in (0, 1):
                                _, _, cd, sg, tg, t1, t2, tn, _ = dirs[d]
                                nc.vector.tensor_tensor(out=t1, in0=sg[:, 4:8], in1=cd, op=ALU.mult)
                                nc.vector.tensor_tensor(out=t2, in0=sg[:, 0:4], in1=tg, op=ALU.mult)
                                nc.vector.tensor_tensor(out=cd, in0=t1, in1=t2, op=ALU.add)
                            for d in (0, 1):
                                nc.scalar.activation(dirs[d][7], dirs[d][2], AF.Tanh)
                            for d in (0, 1):
                                _, hp, _, sg, _, _, _, tn, _ = dirs[d]
                                nc.vector.tensor_tensor(
                                    out=hp[:, u + 1, :], in0=sg[:, 8:12], in1=tn, op=ALU.mult
                                )
                        # end of U steps: batch history copy + slot wrap
                        for d, tgt in ((0, Hs_f), (1, Hs_b)):
                            hp = dirs[d][1]
                            if tgt is not None:
                                nc.vector.tensor_copy(
                                    tgt[:, ds(HC * t0 + HC * U * iv + HC, HC * U)],
                                    hp[:, 1 : U + 1, :].rearrange("p u c -> p (u c)"),
                                )
                            nc.vector.tensor_copy(hp[:, 0, :], hp[:, U, :])
                return dirs[0][2], dirs[1][2], dirs[0][1], dirs[1][1]

            # L1 xp from L0 history tiles (both dirs local)
            def xp_l1(m, Hs_f, Hs_b):
                hfr = Hs_f[:, :].rearrange("p (t c) -> p t c", c=HC)
                hbr = Hs_b[:, :].rearrange("p (t c) -> p t c", c=HC)
                for d in (0, 1):
                    bias = pw.tile([128, GC], f32, tag="bias0", name=f"b1_{m}{d}")
                    nc.sync.dma_start(out=bias, in_=bias_d[(m, 1, d)][:, :])
                    wf = wslab_pool.tile([128, HC * G], bf16, tag="wslab", name=f"w1f_{m}{d}")
                    nc.sync.dma_start(out=wf, in_=wih1_ap[(m, d, "f")])
                    wb = wslab_pool.tile([128, HC * G], bf16, tag="wslab", name=f"w1b_{m}{d}")
                    nc.sync.dma_start(out=wb, in_=wih1_ap[(m, d, "b")])
                    for tb in range(T // NT):
                        s0 = tb * NT  # own-time index of the stored xp block
                        for mc in range(GC):
                            ps = psx_pool.tile([128, NT], f32, tag="psx", name=f"ps1_{m}{d}_{tb}_{mc}")
                            for kc in range(HC):
                                if d == 0:
                                    mv = hfr[:, s0 + 1 : s0 + 1 + NT, kc]
                                else:
                                    mv = hbr[:, s0 + 1 : s0 + 1 + NT, kc]
                                nc.tensor.matmul(
                                    ps,
                                    (wf if d == 0 else wb)[:, kc * G + mc * 128 : kc * G + (mc + 1) * 128],
                                    mv,
                                    start=(kc == 0), stop=False,
                                )
                            for kc in range(HC):
                                # other direction, read in own-time order:
                                # own step s <-> other-storage col T - s
                                if d == 0:
                                    mv = hbr[:, T - s0 : T - s0 - NT : -1, kc]
                                else:
                                    mv = hfr[:, T - s0 : T - s0 - NT : -1, kc]
                                nc.tensor.matmul(
                                    ps,
                                    (wb if d == 0 else wf)[:, kc * G + mc * 128 : kc * G + (mc + 1) * 128],
                                    mv,
                                    start=False, stop=(kc == HC - 1),
                                )
                            st = xst_pool.tile([128, NT], bf16, tag="xst", name=f"st1_{m}{d}_{tb}_{mc}")
                            nc.vector.tensor_scalar(
                                out=st, in0=ps, scalar1=bias[:, mc : mc + 1],
                                scalar2=None, op0=ALU.add,
                            )
                            nc.sync.dma_start(
                                out=xp_dram[(m, 1, d)][:, mc * T + s0 : mc * T + s0 + NT],
                                in_=st,
                            )

            # ================= ENC =========================================
            Hs_e0f = hs_pool.tile([128, HC * (T + 1)], bf16, tag="hs", name="Hs_e0f")
            Hs_e0b = hs_pool.tile([128, HC * (T + 1)], bf16, tag="hs", name="Hs_e0b")
            nc.vector.memset(Hs_e0f[:, 0:4], 0.0)
            nc.vector.memset(Hs_e0b[:, 0:4], 0.0)
            if "scan0" not in skips:
                c_e0f, c_e0b, hp_e0f, hp_e0b = pair_scan("enc", 0, Hs_e0f, Hs_e0b)
            else:
                c_e0f = pw.tile([128, HC], f32, name="dc0f"); nc.vector.memset(c_e0f, 0.0)
                c_e0b = pw.tile([128, HC], f32, name="dc0b"); nc.vector.memset(c_e0b, 0.0)
            if "xp1" not in skips:
                xp_l1("enc", Hs_e0f, Hs_e0b)
            if "scan1" not in skips:
                c_e1f, c_e1b, hp_e1f, hp_e1b = pair_scan("enc", 1, None, None)
            else:
                c_e1f, c_e1b = c_e0f, c_e0b
                hp_e1f = hp_e1b = None

            # ================= finals -> dec init states ====================
            # flat order [l0f l0b l1f l1b]
            fin_h = pw.tile([128, GC], bf16, name="fin_h")
            fin_c = pw.tile([128, GC], bf16, name="fin_c")
            for j, (hsrc, ct) in enumerate(
                (
                    (Hs_e0f[:, HC * T : HC * T + 4], c_e0f),
                    (Hs_e0b[:, HC * T : HC * T + 4], c_e0b),
                    (hp_e1f[:, 0, :] if hp_e1f is not None else Hs_e0f[:, 0:4], c_e1f),
                    (hp_e1b[:, 0, :] if hp_e1b is not None else Hs_e0b[:, 0:4], c_e1b),
                )
            ):
                nc.vector.tensor_copy(fin_h[:, 4 * j : 4 * j + 4], hsrc)
                nc.vector.tensor_copy(fin_c[:, 4 * j : 4 * j + 4], ct)

            init_h = pw.tile([128, GC], f32, name="init_h")
            init_c = pw.tile([128, GC], f32, name="init_c")
            for (wd, bd, fin, out_t) in (
                (e2hT_ap, e2hb_d, fin_h, init_h),
                (e2cT_ap, e2cb_d, fin_c, init_c),
            ):
                eb = pw.tile([128, GC], f32, tag="e2b", name=f"eb_{out_t.name}")
                nc.sync.dma_start(out=eb, in_=bd[:, :])
                wr = wd.rearrange("p (k g) -> p k g", k=GC)
                ps = psx_pool.tile([128, GC], f32, tag="psx", name=f"pse_{out_t.name}")
                for mc in range(GC):
                    eww = win_pool.tile([128, GC, 128], bf16, tag="ww", name=f"eww_{out_t.name}_{mc}")
                    nc.sync.dma_start(out=eww, in_=wr[:, :, mc * 128 : (mc + 1) * 128])
                    for kc in range(GC):
                        nc.tensor.matmul(
                            ps[:, mc : mc + 1],
                            eww[:, kc, :],
                            fin[:, kc : kc + 1],
                            start=(kc == 0), stop=(kc == GC - 1),
                        )
                nc.vector.tensor_tensor(out=out_t, in0=ps, in1=eb, op=ALU.add)
            init_h_bf = pw.tile([128, GC], bf16, name="init_h_bf")
            nc.vector.tensor_copy(init_h_bf, init_h)

            # ================= DEC =========================================
            Hs_d0f = hs_pool.tile([128, HC * (T + 1)], bf16, tag="hs", name="Hs_d0f")
            Hs_d0b = hs_pool.tile([128, HC * (T + 1)], bf16, tag="hs", name="Hs_d0b")
            nc.vector.memset(Hs_d0f[:, 0:4], 0.0)
            nc.vector.memset(Hs_d0b[:, 0:4], 0.0)
            if "scan2" not in skips:
                c_d0f, c_d0b, _, _ = pair_scan("dec", 0, Hs_d0f, Hs_d0b, init_h_bf, init_c)
            if "xp3" not in skips:
                xp_l1("dec", Hs_d0f, Hs_d0b)
            Hs_d1f = hs_pool.tile([128, HC * (T + 1)], bf16, tag="hs", name="Hs_d1f")
            Hs_d1b = hs_pool.tile([128, HC * (T + 1)], bf16, tag="hs", name="Hs_d1b")
            nc.vector.memset(Hs_d1f[:, 0:4], 0.0)
            nc.vector.memset(Hs_d1b[:, 0:4], 0.0)
            if "scan3" not in skips:
                c_d1f, c_d1b, _, _ = pair_scan("dec", 1, Hs_d1f, Hs_d1b, init_h_bf, init_c)

            # ================= feats =======================================
            h2tf = pw.tile([128, HC * K], bf16, name="h2tf")
            nc.sync.dma_start(out=h2tf, in_=h2tT_f_ap)
            h2tb = pw.tile([128, HC * K], bf16, name="h2tb")
            nc.sync.dma_start(out=h2tb, in_=h2tT_b_ap)
            h2tbias = pw.tile([K, 1], f32, name="h2tbias")
            nc.sync.dma_start(out=h2tbias, in_=h2tb_d[:, :])
            feats = pw.tile([K, T], f32, name="feats")
            d1fr = Hs_d1f[:, :].rearrange("p (t c) -> p t c", c=HC)
            d1br = Hs_d1b[:, :].rearrange("p (t c) -> p t c", c=HC)
            for tb in range(T // NT):
                t0 = tb * NT
                ps = psx_pool.tile([K, NT], f32, tag="psx", name=f"psf_{tb}")
                for kc in range(HC):
                    nc.tensor.matmul(
                        ps, h2tf[:, kc * K : (kc + 1) * K],
                        d1fr[:, t0 + 1 : t0 + 1 + NT, kc],
                        start=(kc == 0), stop=False,
                    )
                for kc in range(HC):
                    nc.tensor.matmul(
                        ps, h2tb[:, kc * K : (kc + 1) * K],
                        d1br[:, T - t0 : T - t0 - NT : -1, kc],
                        start=False, stop=(kc == HC - 1),
                    )
                nc.vector.tensor_scalar(
                    out=feats[:, t0 : t0 + NT], in0=ps, scalar1=h2tbias,
                    scalar2=None, op0=ALU.add,
                )
            nc.sync.dma_start(out=feats_out[:, :], in_=feats)

            # ================= CRF =========================================
            expF = feats
            nc.scalar.activation(expF, feats, AF.Exp)
            EexpT = pw.tile([K, K], f32, name="EexpT")
            nc.sync.dma_start(out=EexpT, in_=EexpT_d[:, :])
            Eexp = pw.tile([K, K], f32, name="Eexp")
            nc.sync.dma_start(out=Eexp, in_=Eexp_d[:, :])
            ones48 = pw.tile([K, K], f32, name="ones48")
            nc.sync.dma_start(out=ones48, in_=ones48_d[:, :])
            alpha = pw.tile([K, 1], f32, name="alpha")
            nc.sync.dma_start(out=alpha, in_=alpha0_d[:, :])
            beta = pw.tile([K, 1], f32, name="beta")
            nc.sync.dma_start(out=beta, in_=betaT_d[:, :])
            gam = pw.tile([K, 1], f32, name="gam")
            rsA = pw.tile([K, 1], f32, name="rsA")
            rsB = pw.tile([K, 1], f32, name="rsB")
            Ssb = pw.tile([1, NSA + NSB + 1], f32, name="Ssb")

            # alpha over t = 0..TH-1 ; beta over t = T-1..TH (gamma scaling).
            # beta tile holds beta_t; step i: gam = e_{T-1-i} * beta;
            # beta' = E^T @ gam. After TH steps beta = beta_{TH-1}.
            # ACT scalar operands cannot take register offsets -> prefetch the
            # body's expF columns into fixed tiles each iteration.
            ecA = pw.tile([K, RN], f32, name="ecA")
            ecB = pw.tile([K, RN], f32, name="ecB")
            with (tc.For_i(0, TH // RN) if "crf" not in skips else tc.For_i(0, 1)) as iv:
                nc.vector.tensor_copy(ecA, expF[:, ds(RN * iv, RN)])
                nc.vector.tensor_copy(ecB, expF[:, ds(T - RN - RN * iv, RN)])
                for u in range(RN):
                    # alpha: psA = E @ alpha ; alpha = e_t * psA
                    psA = pss_pool.tile([K, 1], f32, tag="ps0", name=f"psA_{u}")
                    nc.tensor.matmul(psA, EexpT, alpha, start=True, stop=True)
                    # beta: gam = e_{T-1-i} * beta (ACT), then psB = E^T @ gam
                    nc.scalar.activation(
                        gam, beta, AF.Copy, scale=ecB[:, RN - 1 - u : RN - u],
                    )
                    nc.scalar.activation(
                        alpha, psA, AF.Copy, scale=ecA[:, u : u + 1]
                    )
                    psB = pss_pool.tile([K, 1], f32, tag="ps1", name=f"psB_{u}")
                    nc.tensor.matmul(psB, Eexp, gam, start=True, stop=True)
                    nc.vector.tensor_copy(beta, psB)
                # renorm both streams; store norms
                psSA = pss_pool.tile([K, 1], f32, tag="ps0", name="psSA")
                nc.tensor.matmul(psSA, ones48, alpha, start=True, stop=True)
                nc.vector.reciprocal(rsA, psSA)
                nc.vector.tensor_tensor(out=alpha, in0=alpha, in1=rsA, op=ALU.mult)
                nc.vector.tensor_copy(Ssb[:, ds(iv, 1)], psSA[0:1, :])
                psSB = pss_pool.tile([K, 1], f32, tag="ps1", name="psSB")
                nc.tensor.matmul(psSB, ones48, beta, start=True, stop=True)
                nc.vector.reciprocal(rsB, psSB)
                nc.vector.tensor_tensor(out=beta, in0=beta, in1=rsB, op=ALU.mult)
                nc.vector.tensor_copy(Ssb[:, ds(NSA + iv, 1)], psSB[0:1, :])
            # final: dot(alpha_{TH-1}, beta_{TH-1})
            psZ = pss_pool.tile([1, 1], f32, tag="ps0", name="psZ")
            nc.tensor.matmul(psZ, alpha, beta, start=True, stop=True)
            nc.vector.tensor_copy(Ssb[:, NSA + NSB : NSA + NSB + 1], psZ)
            nc.sync.dma_start(out=snorm_out[:, :], in_=Ssb)
    nc.compile()
    return nc


# ----------------------------------------------------------------------------
# entry point
# ----------------------------------------------------------------------------

def _postprocess(r0, inputs):
    feats = r0["feats"].astype(np.float64)  # [K, T]
    sn = r0["snorm"].astype(np.float64)[0]
    Z = np.log(sn).sum()

    tags = np.asarray(inputs["tags"]).astype(np.int64)
    trans = np.asarray(inputs["transitions"]).astype(np.float64)
    ext = np.concatenate([[START_IDX], tags])
    score = trans[ext[1:], ext[:-1]].sum() + feats[tags, np.arange(T)].sum()
    score += trans[END_IDX, tags[-1]]
    return np.float32(Z - score)


def kernel(**inputs) -> np.ndarray:
    if "nc" not in _CACHE:
        _CACHE["nc"] = build()
    nc = _CACHE["nc"]
    in_map = _prep(inputs)
    res = run_bass_kernel_spmd(nc, [in_map], [0])
    return _postprocess(res.results[0], inputs)


# revision 8
# speedup vs baseline: 1.2969x; 1.2969x over previous
"""BiLSTM Enc-Dec + CRF NLL loss on ONE Trainium2 core (zero collectives).

Design (from microbenchmarking this hardware):
- Small matmuls with register-offset (loop-var) access patterns cost ~300ns
  each; with constant offsets ~15ns. So the recurrent h@W_hh matvec keeps h
  in a (U+1)-slot ping-pong buffer indexed by the Python-unrolled step index:
  all 64 PE matmuls per step use constant APs. The time-indexed history write
  (needed by the next layer's input projection) is a batched DVE copy per
  unrolled body (register offsets are cheap on DVE).
- Collectives cost ~41 ms fixed per execution on this stack, so everything
  runs on core 0: the fwd and bwd direction scans of each layer are
  interleaved step-by-step on one core, which hides most of each scan's
  serial chain latency in the other's engine gaps.
- The per-step xp (input projection) add is folded into PSUM via an identity
  matmul (start=True) so activations read gates straight from PSUM.
- CRF partition function: linear domain with renorm every 8 steps; split
  into a forward alpha recursion over t=0..1023 and a backward beta
  recursion over t=2047..1024, interleaved on the same engines; the host
  sums the logs of the stored norms in float64.
"""

import sys

sys.path.insert(0, "/opt/trn_rl_repo")

import numpy as np
import ml_dtypes

import concourse.bacc as bacc
import concourse.mybir as mybir
from concourse.bass import ds
from concourse.tile import TileContext
from concourse.bass_utils import run_bass_kernel_spmd

T = 2048
ELMO = 1024
H = 512
POS = 64
K = 48
S = 50
L = 2
NEG = -10000.0
START_IDX, END_IDX = 0, 1

Din0 = ELMO + POS  # 1088
K0C = 9  # ceil(1088/128)
HC = 4
G = 4 * H  # 2048
GC = 16
U = 16  # unrolled steps per hardware-loop body
CH = 128  # steps per xp window
NT = 512  # time-block for bulk matmuls
RN = 8  # CRF renorm cadence
TH = T // 2  # alpha/beta split point

bf16 = mybir.dt.bfloat16
f32 = mybir.dt.float32
AF = mybir.ActivationFunctionType
ALU = mybir.AluOpType

_CACHE = {}

STAGES = [("enc", 0), ("enc", 1), ("dec", 0), ("dec", 1)]

# all big bf16 inputs are packed into ONE [128, MEGA_COLS] tensor: several
# multi-MB separate input tensors cost ~10 ms EACH per execution on this
# stack, while one giant tensor (or many tiny ones) is free.
MEGA = [("embT", K0C * T), ("ident", 128)]
for _m in ("enc", "dec"):
    for _l in (0, 1):
        for _d in (0, 1):
            MEGA.append((f"whhT_{_m}{_l}_{_d}", HC * G))
    for _d in (0, 1):
        MEGA.append((f"wih0T_{_m}_{_d}", K0C * G))
        for _s in ("f", "b"):
            MEGA.append((f"wih1T_{_m}_{_d}_{_s}", HC * G))
MEGA += [("e2hT", GC * G), ("e2cT", GC * G), ("h2tT_f", HC * K), ("h2tT_b", HC * K)]
MEGA_OFF = {}
_o = 0
for _n, _w in MEGA:
    MEGA_OFF[_n] = _o
    _o += _w
MEGA_COLS = _o


# ----------------------------------------------------------------------------
# host-side weight preparation
# ----------------------------------------------------------------------------

def _perm_gates(a):
    """reorder gate rows [i,f,g,o] -> [i,f,o,g] along axis 0 (size 4H)."""
    return np.concatenate([a[0:H], a[H : 2 * H], a[3 * H : 4 * H], a[2 * H : 3 * H]], 0)


def _tile_kT(wT, nk):
    """[Ktot, M] -> [128, nk*M] with col kc*M + m = wT[kc*128 + p, m]."""
    Ktot, M = wT.shape
    assert Ktot == nk * 128
    return np.ascontiguousarray(wT.reshape(nk, 128, M).transpose(1, 0, 2).reshape(128, nk * M))


def _prep(inputs):
    f = np.float32
    ins = {}
    sentence = inputs["sentence"].astype(f)
    pos_emb = inputs["pos_emb"].astype(f)
    speech = np.asarray(inputs["speech_tags"]).astype(np.int64)
    embeds = np.concatenate([sentence, pos_emb[speech]], axis=1)  # (T, 1088)
    embT = np.zeros((K0C * 128, T), f)
    embT[:Din0] = embeds.T
    ins["embT"] = _tile_kT(embT, K0C).astype(ml_dtypes.bfloat16)
    ins["ident"] = np.eye(128).astype(ml_dtypes.bfloat16)

    for m in ("enc", "dec"):
        for l in (0, 1):
            for d in (0, 1):
                whh = _perm_gates(inputs[f"{m}_w_hh{l}"][d].astype(f))
                ins[f"whhT_{m}{l}_{d}"] = _tile_kT(
                    np.ascontiguousarray(whh.T), HC
                ).astype(ml_dtypes.bfloat16)
                b = _perm_gates(
                    (inputs[f"{m}_b_ih{l}"][d] + inputs[f"{m}_b_hh{l}"][d]).astype(f)
                )
                ins[f"bias_{m}{l}_{d}"] = np.ascontiguousarray(
                    b.reshape(GC, 128).T
                ).astype(f)
        for d in (0, 1):
            wih0 = _perm_gates(inputs[f"{m}_w_ih0"][d].astype(f))  # (2048, 1088)
            w0T = np.zeros((K0C * 128, G), f)
            w0T[:Din0] = wih0.T
            ins[f"wih0T_{m}_{d}"] = _tile_kT(w0T, K0C).astype(ml_dtypes.bfloat16)
            wih1 = _perm_gates(inputs[f"{m}_w_ih1"][d].astype(f))  # (2048, 1024)
            wf = wih1[:, 0:H]  # multiplies fwd-dir L0 outputs
            wb = wih1[:, H : 2 * H]  # multiplies bwd-dir L0 outputs
            ins[f"wih1T_{m}_{d}_f"] = _tile_kT(
                np.ascontiguousarray(wf.T), HC
            ).astype(ml_dtypes.bfloat16)
            ins[f"wih1T_{m}_{d}_b"] = _tile_kT(
                np.ascontiguousarray(wb.T), HC
            ).astype(ml_dtypes.bfloat16)

    # e2h/e2c: natural order both sides. out rows = [dl0f dl0b dl1f dl1b],
    # in cols = [el0f el0b el1f el1b] (PyTorch flat order of (2L, H) states).
    for nm in ("e2h", "e2c"):
        w = inputs[f"{nm}_w"].astype(f)  # (2048, 2048)
        ins[f"{nm}T"] = _tile_kT(np.ascontiguousarray(w.T), GC).astype(ml_dtypes.bfloat16)
        ins[f"{nm}_b"] = np.ascontiguousarray(
            inputs[f"{nm}_b"].astype(f).reshape(GC, 128).T
        ).astype(f)

    h2t = inputs["h2t_w"].astype(f)  # (K, 1024)
    ins["h2tT_f"] = _tile_kT(np.ascontiguousarray(h2t[:, 0:H].T), HC).astype(
        ml_dtypes.bfloat16
    )
    ins["h2tT_b"] = _tile_kT(np.ascontiguousarray(h2t[:, H:].T), HC).astype(
        ml_dtypes.bfloat16
    )
    ins["h2t_b"] = inputs["h2t_b"].astype(f).reshape(K, 1)

    trans = inputs["transitions"].astype(np.float64)
    E = np.exp(trans).astype(f)  # E[next, prev]
    ins["EexpT"] = np.ascontiguousarray(E.T)  # lhsT for alpha: out = E @ x
    ins["Eexp"] = np.ascontiguousarray(E)  # lhsT for beta: out = E^T @ x
    ins["betaT"] = np.ascontiguousarray(E[END_IDX].reshape(K, 1))  # exp(trans[END])
    a0 = np.zeros((K, 1), f)
    a0[START_IDX, 0] = 1.0
    ins["alpha0"] = a0
    ins["ones48"] = np.ones((K, K), f)
    mega = np.empty((128, MEGA_COLS), ml_dtypes.bfloat16)
    for n, w in MEGA:
        mega[:, MEGA_OFF[n] : MEGA_OFF[n] + w] = ins.pop(n)
    ins["mega"] = mega
    return ins


# ----------------------------------------------------------------------------
# device program
# ----------------------------------------------------------------------------

def build():
    import os
    skips = set(os.environ.get("BK_SKIP", "").split(","))
    nc = bacc.Bacc("TRN2", target_bir_lowering=False, num_devices=1)

    def din(name, shape, dt=bf16):
        return nc.dram_tensor(name, shape, dt, kind="ExternalInput")

    mega_d = din("mega", [128, MEGA_COLS])

    def mega_ap(name):
        o = MEGA_OFF[name]
        return mega_d[:, o : o + dict(MEGA)[name]]

    embT_ap = mega_ap("embT")
    ident_ap = mega_ap("ident")
    whh_ap = {
        (m, l, d): mega_ap(f"whhT_{m}{l}_{d}")
        for m in ("enc", "dec") for l in (0, 1) for d in (0, 1)
    }
    bias_d = {
        (m, l, d): din(f"bias_{m}{l}_{d}", [128, GC], f32)
        for m in ("enc", "dec") for l in (0, 1) for d in (0, 1)
    }
    wih0_ap = {(m, d): mega_ap(f"wih0T_{m}_{d}") for m in ("enc", "dec") for d in (0, 1)}
    wih1_ap = {
        (m, d, s): mega_ap(f"wih1T_{m}_{d}_{s}")
        for m in ("enc", "dec") for d in (0, 1) for s in ("f", "b")
    }
    e2hT_ap = mega_ap("e2hT")
    e2cT_ap = mega_ap("e2cT")
    e2hb_d = din("e2h_b", [128, GC], f32)
    e2cb_d = din("e2c_b", [128, GC], f32)
    h2tT_f_ap = mega_ap("h2tT_f")
    h2tT_b_ap = mega_ap("h2tT_b")
    h2tb_d = din("h2t_b", [K, 1], f32)
    EexpT_d = din("EexpT", [K, K], f32)
    Eexp_d = din("Eexp", [K, K], f32)
    betaT_d = din("betaT", [K, 1], f32)
    alpha0_d = din("alpha0", [K, 1], f32)
    ones48_d = din("ones48", [K, K], f32)

    feats_out = nc.dram_tensor("feats", [K, T], f32, kind="ExternalOutput")
    NSA = TH // RN  # 128 alpha norms
    NSB = (T - TH) // RN  # 128 beta norms
    snorm_out = nc.dram_tensor("snorm", [1, NSA + NSB + 1], f32, kind="ExternalOutput")

    # internal DRAM xp buffers: layer 1 reuses layer 0's (its xp is fully
    # consumed by the L0 scan before xp_l1 rewrites the buffer)
    xp_dram = {}
    for m in ("enc", "dec"):
        for d in (0, 1):
            buf = nc.dram_tensor(f"xp_{m}_{d}", [128, GC * T], bf16)
            xp_dram[(m, 0, d)] = buf
            xp_dram[(m, 1, d)] = buf

    with TileContext(nc) as tc:
        with (
            tc.tile_pool(name="pw", bufs=1) as pw,
            tc.tile_pool(name="wslab", bufs=2) as wslab_pool,  # whh / wih1 slabs
            tc.tile_pool(name="hs", bufs=3) as hs_pool,
            tc.tile_pool(name="win", bufs=2) as win_pool,  # streamed emb windows
            tc.tile_pool(name="w0s", bufs=2) as w0s_pool,  # wih0 half-slab windows
            tc.tile_pool(name="xw", bufs=2) as xw_pool,  # xp scan windows
            tc.tile_pool(name="xst", bufs=6) as xst_pool,  # xp store staging
            tc.tile_pool(name="psx", bufs=2, space="PSUM") as psx_pool,
            tc.tile_pool(name="pss", bufs=3, space="PSUM") as pss_pool,
        ):
            ident = pw.tile([128, 128], bf16, name="ident")
            nc.sync.dma_start(out=ident, in_=ident_ap)

            # ================= P0: layer-0 xp for all 4 (model, dir) =========
            embr = embT_ap.rearrange("p (k t) -> p k t", k=K0C)
            for m in ("enc", "dec") if "p0" not in skips else ():
                for d in (0, 1):
                    bias = pw.tile([128, GC], f32, tag="bias0", name=f"b0_{m}{d}")
                    nc.sync.dma_start(out=bias, in_=bias_d[(m, 0, d)][:, :])
                    w0r = wih0_ap[(m, d)].rearrange("p (k g) -> p k g", k=K0C)
                    w0h = []
                    for half in (0, 1):
                        w0t = w0s_pool.tile(
                            [128, K0C, G // 2], bf16, tag="w0h", name=f"w0_{m}{d}_{half}"
                        )
                        nc.sync.dma_start(
                            out=w0t, in_=w0r[:, :, half * (G // 2) : (half + 1) * (G // 2)]
                        )
                        w0h.append(w0t)
                    for tb in range(T // NT):
                        t0 = tb * NT
                        ew = win_pool.tile([128, K0C, NT], bf16, tag="ew", name=f"ew_{m}{d}_{tb}")
                        nc.sync.dma_start(out=ew, in_=embr[:, :, t0 : t0 + NT])
                        if d == 0:
                            mv = ew[:, :, :]
                        else:
                            # bwd dir: reversed time; psum col j = bwd-step
                            # s = (T - t0 - NT) + j
                            mv = ew[:, :, NT - 1 :: -1]
                        s0 = t0 if d == 0 else T - t0 - NT
                        for mc in range(GC):
                            wt = w0h[mc // 8]
                            mo = (mc % 8) * 128
                            ps = psx_pool.tile([128, NT], f32, tag="psx", name=f"ps0_{m}{d}_{tb}_{mc}")
                            for kc in range(K0C):
                                nc.tensor.matmul(
                                    ps, wt[:, kc, mo : mo + 128], mv[:, kc, :],
                                    start=(kc == 0), stop=(kc == K0C - 1),
                                )
                            st = xst_pool.tile([128, NT], bf16, tag="xst", name=f"st0_{m}{d}_{tb}_{mc}")
                            nc.vector.tensor_scalar(
                                out=st, in0=ps, scalar1=bias[:, mc : mc + 1],
                                scalar2=None, op0=ALU.add,
                            )
                            nc.sync.dma_start(
                                out=xp_dram[(m, 0, d)][:, mc * T + s0 : mc * T + s0 + NT],
                                in_=st,
                            )

            # ================= scan machinery ================================
            def pair_scan(m, l, Hs_f, Hs_b, init_h=None, init_c=None):
                """Interleaved fwd/bwd scan for stage (m, l). Hs_* are
                [128, HC*(T+1)] bf16 history tiles. init_h/init_c are
                ([128,16] bf16, [128,16] f32) tiles; columns 4*scan_idx.. hold
                the state for scan (l, d). Returns (c_f, c_b) f32 tiles."""
                dirs = []
                for d in (0, 1):
                    W = wslab_pool.tile([128, HC * G], bf16, tag="wslab", name=f"whh_{m}{l}_{d}")
                    nc.sync.dma_start(out=W, in_=whh_ap[(m, l, d)])
                    hp = pw.tile([128, U + 1, HC], bf16, tag=f"hp{d}", name=f"hp_{m}{l}_{d}")
                    cd = pw.tile([128, HC], f32, tag=f"c{d}", name=f"c_{m}{l}_{d}")
                    si = 2 * l + d
                    if init_h is None:
                        nc.vector.memset(hp[:, 0, :], 0.0)
                        nc.vector.memset(cd, 0.0)
                    else:
                        nc.vector.tensor_copy(hp[:, 0, :], init_h[:, 4 * si : 4 * si + 4])
                        nc.vector.tensor_copy(cd, init_c[:, 4 * si : 4 * si + 4])
                    sg = pw.tile([128, 12], f32, tag=f"sg{d}", name=f"sg_{m}{l}_{d}")
                    tg = pw.tile([128, 4], f32, tag=f"tg{d}", name=f"tg_{m}{l}_{d}")
                    t1 = pw.tile([128, 4], f32, tag=f"t1{d}", name=f"t1_{m}{l}_{d}")
                    t2 = pw.tile([128, 4], f32, tag=f"t2{d}", name=f"t2_{m}{l}_{d}")
                    tn = pw.tile([128, 4], f32, tag=f"tn{d}", name=f"tn_{m}{l}_{d}")
                    dirs.append([W, hp, cd, sg, tg, t1, t2, tn, None])

                xpr = {
                    d: xp_dram[(m, l, d)][:, :].rearrange("p (g t) -> p g t", g=GC)
                    for d in (0, 1)
                }
                for w in range(T // CH):
                    t0 = w * CH
                    for d in (0, 1):
                        xwt = xw_pool.tile(
                            [128, GC, CH], bf16, tag=f"xw{d}", name=f"xw_{m}{l}_{d}_{w}"
                        )
                        nc.sync.dma_start(out=xwt, in_=xpr[d][:, :, t0 : t0 + CH])
                        dirs[d][8] = xwt
                    xc = [
                        pw.tile([128, GC, U], bf16, tag=f"xc{d}", name=f"xc_{m}{l}_{d}_{w}")
                        for d in (0, 1)
                    ]
                    with tc.For_i(0, CH // U) as iv:
                        for d in (0, 1):
                            nc.gpsimd.tensor_copy(
                                xc[d],
                                dirs[d][8][:, :, ds(U * iv, U)],
                            )
                        for u in range(U):
                            pps = []
                            for d in (0, 1):
                                W, hp = dirs[d][0], dirs[d][1]
                                ps = pss_pool.tile(
                                    [128, GC], f32, tag=f"ps{d}", name=f"ps_{m}{l}_{d}_{u}"
                                )
                                nc.tensor.matmul(
                                    ps, ident, xc[d][:, :, u], start=True, stop=False,
                                    skip_group_check=True,
                                )
                                for mc in range(GC):
                                    for kc in range(HC):
                                        nc.tensor.matmul(
                                            ps[:, mc : mc + 1],
                                            W[:, kc * G + mc * 128 : kc * G + (mc + 1) * 128],
                                            hp[:, u, kc : kc + 1],
                                            start=False,
                                            stop=(kc == HC - 1),
                                            skip_group_check=True,
                                        )
                                pps.append(ps)
                            for d in (0, 1):
                                sg, tg = dirs[d][3], dirs[d][4]
                                nc.scalar.activation(sg, pps[d][:, 0:12], AF.Sigmoid)
                                nc.scalar.activation(tg, pps[d][:, 12:16], AF.Tanh)
                            for d in (0, 1):
                                _, _, cd, sg, tg, t1, t2, tn, _ = dirs[d]
                                nc.vector.tensor_tensor(out=t1, in0=sg[:, 4:8], in1=cd, op=ALU.mult)
                                nc.vector.tensor_tensor(out=t2, in0=sg[:, 0:4], in1=tg, op=ALU.mult)
                                nc.vector.tensor_tensor(out=cd, in0=t1, in1=t2, op=ALU.add)
                            for d in (0, 1):
                                nc.scalar.activation(dirs[d][7], dirs[d][2], AF.Tanh)
                            for d in (0, 1):
                                _, hp, _, sg, _, _, _, tn, _ = dirs[d]
                                nc.vector.tensor_tensor(
                                    out=hp[:, u + 1, :], in0=sg[:, 8:12], in1=tn, op=ALU.mult
                                )
                        # end of U steps: batch history copy + slot wrap
                        for d, tgt in ((0, Hs_f), (1, Hs_b)):
                            hp = dirs[d][1]
                            if tgt is not None:
                                nc.gpsimd.tensor_copy(
                                    tgt[:, ds(HC * t0 + HC * U * iv + HC, HC * U)],
                                    hp[:, 1 : U + 1, :].rearrange("p u c -> p (u c)"),
                                )
                            nc.vector.tensor_copy(hp[:, 0, :], hp[:, U, :])
                return dirs[0][2], dirs[1][2], dirs[0][1], dirs[1][1]

            # L1 xp from L0 history tiles (both dirs local)
            def xp_l1(m, Hs_f, Hs_b):
                hfr = Hs_f[:, :].rearrange("p (t c) -> p t c", c=HC)
                hbr = Hs_b[:, :].rearrange("p (t c) -> p t c", c=HC)
                for d in (0, 1):
                    bias = pw.tile([128, GC], f32, tag="bias0", name=f"b1_{m}{d}")
                    nc.sync.dma_start(out=bias, in_=bias_d[(m, 1, d)][:, :])
                    wf = wslab_pool.tile([128, HC * G], bf16, tag="wslab", name=f"w1f_{m}{d}")
                    nc.sync.dma_start(out=wf, in_=wih1_ap[(m, d, "f")])
                    wb = wslab_pool.tile([128, HC * G], bf16, tag="wslab", name=f"w1b_{m}{d}")
                    nc.sync.dma_start(out=wb, in_=wih1_ap[(m, d, "b")])
                    for tb in range(T // NT):
                        s0 = tb * NT  # own-time index of the stored xp block
                        for mc in range(GC):
                            ps = psx_pool.tile([128, NT], f32, tag="psx", name=f"ps1_{m}{d}_{tb}_{mc}")
                            for kc in range(HC):
                                if d == 0:
                                    mv = hfr[:, s0 + 1 : s0 + 1 + NT, kc]
                                else:
                                    mv = hbr[:, s0 + 1 : s0 + 1 + NT, kc]
                                nc.tensor.matmul(
                                    ps,
                                    (wf if d == 0 else wb)[:, kc * G + mc * 128 : kc * G + (mc + 1) * 128],
                                    mv,
                                    start=(kc == 0), stop=False,
                                )
                            for kc in range(HC):
                                # other direction, read in own-time order:
                                # own step s <-> other-storage col T - s
                                if d == 0:
                                    mv = hbr[:, T - s0 : T - s0 - NT : -1, kc]
                                else:
                                    mv = hfr[:, T - s0 : T - s0 - NT : -1, kc]
                                nc.tensor.matmul(
                                    ps,
                                    (wb if d == 0 else wf)[:, kc * G + mc * 128 : kc * G + (mc + 1) * 128],
                                    mv,
                                    start=False, stop=(kc == HC - 1),
                                )
                            st = xst_pool.tile([128, NT], bf16, tag="xst", name=f"st1_{m}{d}_{tb}_{mc}")
                            nc.vector.tensor_scalar(
                                out=st, in0=ps, scalar1=bias[:, mc : mc + 1],
                                scalar2=None, op0=ALU.add,
                            )
                            nc.sync.dma_start(
                                out=xp_dram[(m, 1, d)][:, mc * T + s0 : mc * T + s0 + NT],
                                in_=st,
                            )

            # ================= ENC =========================================
            Hs_e0f = hs_pool.tile([128, HC * (T + 1)], bf16, tag="hs", name="Hs_e0f")
            Hs_e0b = hs_pool.tile([128, HC * (T + 1)], bf16, tag="hs", name="Hs_e0b")
            nc.vector.memset(Hs_e0f[:, 0:4], 0.0)
            nc.vector.memset(Hs_e0b[:, 0:4], 0.0)
            if "scan0" not in skips:
                c_e0f, c_e0b, hp_e0f, hp_e0b = pair_scan("enc", 0, Hs_e0f, Hs_e0b)
            else:
                c_e0f = pw.tile([128, HC], f32, name="dc0f"); nc.vector.memset(c_e0f, 0.0)
                c_e0b = pw.tile([128, HC], f32, name="dc0b"); nc.vector.memset(c_e0b, 0.0)
            if "xp1" not in skips:
                xp_l1("enc", Hs_e0f, Hs_e0b)
            if "scan1" not in skips:
                c_e1f, c_e1b, hp_e1f, hp_e1b = pair_scan("enc", 1, None, None)
            else:
                c_e1f, c_e1b = c_e0f, c_e0b
                hp_e1f = hp_e1b = None

            # ================= finals -> dec init states ====================
            # flat order [l0f l0b l1f l1b]
            fin_h = pw.tile([128, GC], bf16, name="fin_h")
            fin_c = pw.tile([128, GC], bf16, name="fin_c")
            for j, (hsrc, ct) in enumerate(
                (
                    (Hs_e0f[:, HC * T : HC * T + 4], c_e0f),
                    (Hs_e0b[:, HC * T : HC * T + 4], c_e0b),
                    (hp_e1f[:, 0, :] if hp_e1f is not None else Hs_e0f[:, 0:4], c_e1f),
                    (hp_e1b[:, 0, :] if hp_e1b is not None else Hs_e0b[:, 0:4], c_e1b),
                )
            ):
                nc.vector.tensor_copy(fin_h[:, 4 * j : 4 * j + 4], hsrc)
                nc.vector.tensor_copy(fin_c[:, 4 * j : 4 * j + 4], ct)

            init_h = pw.tile([128, GC], f32, name="init_h")
            init_c = pw.tile([128, GC], f32, name="init_c")
            for (wd, bd, fin, out_t) in (
                (e2hT_ap, e2hb_d, fin_h, init_h),
                (e2cT_ap, e2cb_d, fin_c, init_c),
            ):
                eb = pw.tile([128, GC], f32, tag="e2b", name=f"eb_{out_t.name}")
                nc.sync.dma_start(out=eb, in_=bd[:, :])
                wr = wd.rearrange("p (k g) -> p k g", k=GC)
                ps = psx_pool.tile([128, GC], f32, tag="psx", name=f"pse_{out_t.name}")
                for mc in range(GC):
                    eww = win_pool.tile([128, GC, 128], bf16, tag="ww", name=f"eww_{out_t.name}_{mc}")
                    nc.sync.dma_start(out=eww, in_=wr[:, :, mc * 128 : (mc + 1) * 128])
                    for kc in range(GC):
                        nc.tensor.matmul(
                            ps[:, mc : mc + 1],
                            eww[:, kc, :],
                            fin[:, kc : kc + 1],
                            start=(kc == 0), stop=(kc == GC - 1),
                        )
                nc.vector.tensor_tensor(out=out_t, in0=ps, in1=eb, op=ALU.add)
            init_h_bf = pw.tile([128, GC], bf16, name="init_h_bf")
            nc.vector.tensor_copy(init_h_bf, init_h)

            # ================= DEC =========================================
            Hs_d0f = hs_pool.tile([128, HC * (T + 1)], bf16, tag="hs", name="Hs_d0f")
            Hs_d0b = hs_pool.tile([128, HC * (T + 1)], bf16, tag="hs", name="Hs_d0b")
            nc.vector.memset(Hs_d0f[:, 0:4], 0.0)
            nc.vector.memset(Hs_d0b[:, 0:4], 0.0)
            if "scan2" not in skips:
                c_d0f, c_d0b, _, _ = pair_scan("dec", 0, Hs_d0f, Hs_d0b, init_h_bf, init_c)
            if "xp3" not in skips:
                xp_l1("dec", Hs_d0f, Hs_d0b)
            Hs_d1f = hs_pool.tile([128, HC * (T + 1)], bf16, tag="hs", name="Hs_d1f")
            Hs_d1b = hs_pool.tile([128, HC * (T + 1)], bf16, tag="hs", name="Hs_d1b")
            nc.vector.memset(Hs_d1f[:, 0:4], 0.0)
            nc.vector.memset(Hs_d1b[:, 0:4], 0.0)
            if "scan3" not in skips:
                c_d1f, c_d1b, _, _ = pair_scan("dec", 1, Hs_d1f, Hs_d1b, init_h_bf, init_c)

            # ================= feats =======================================
            h2tf = pw.tile([128, HC * K], bf16, name="h2tf")
            nc.sync.dma_start(out=h2tf, in_=h2tT_f_ap)
            h2tb = pw.tile([128, HC * K], bf16, name="h2tb")
            nc.sync.dma_start(out=h2tb, in_=h2tT_b_ap)
            h2tbias = pw.tile([K, 1], f32, name="h2tbias")
            nc.sync.dma_start(out=h2tbias, in_=h2tb_d[:, :])
            feats = pw.tile([K, T], f32, name="feats")
            d1fr = Hs_d1f[:, :].rearrange("p (t c) -> p t c", c=HC)
            d1br = Hs_d1b[:, :].rearrange("p (t c) -> p t c", c=HC)
            for tb in range(T // NT):
                t0 = tb * NT
                ps = psx_pool.tile([K, NT], f32, tag="psx", name=f"psf_{tb}")
                for kc in range(HC):
                    nc.tensor.matmul(
                        ps, h2tf[:, kc * K : (kc + 1) * K],
                        d1fr[:, t0 + 1 : t0 + 1 + NT, kc],
                        start=(kc == 0), stop=False,
                    )
                for kc in range(HC):
                    nc.tensor.matmul(
                        ps, h2tb[:, kc * K : (kc + 1) * K],
                        d1br[:, T - t0 : T - t0 - NT : -1, kc],
                        start=False, stop=(kc == HC - 1),
                    )
                nc.vector.tensor_scalar(
                    out=feats[:, t0 : t0 + NT], in0=ps, scalar1=h2tbias,
                    scalar2=None, op0=ALU.add,
                )
            nc.sync.dma_start(out=feats_out[:, :], in_=feats)

            # ================= CRF =========================================
            expF = feats
            nc.scalar.activation(expF, feats, AF.Exp)
            EexpT = pw.tile([K, K], f32, name="EexpT")
            nc.sync.dma_start(out=EexpT, in_=EexpT_d[:, :])
            Eexp = pw.tile([K, K], f32, name="Eexp")
            nc.sync.dma_start(out=Eexp, in_=Eexp_d[:, :])
            ones48 = pw.tile([K, K], f32, name="ones48")
            nc.sync.dma_start(out=ones48, in_=ones48_d[:, :])
            alpha = pw.tile([K, 1], f32, name="alpha")
            nc.sync.dma_start(out=alpha, in_=alpha0_d[:, :])
            beta = pw.tile([K, 1], f32, name="beta")
            nc.sync.dma_start(out=beta, in_=betaT_d[:, :])
            gam = pw.tile([K, 1], f32, name="gam")
            rsA = pw.tile([K, 1], f32, name="rsA")
            rsB = pw.tile([K, 1], f32, name="rsB")
            Ssb = pw.tile([1, NSA + NSB + 1], f32, name="Ssb")

            # alpha over t = 0..TH-1 ; beta over t = T-1..TH (gamma scaling).
            # beta tile holds beta_t; step i: gam = e_{T-1-i} * beta;
            # beta' = E^T @ gam. After TH steps beta = beta_{TH-1}.
            # ACT scalar operands cannot take register offsets -> prefetch the
            # body's expF columns into fixed tiles each iteration.
            ecA = pw.tile([K, RN], f32, name="ecA")
            ecB = pw.tile([K, RN], f32, name="ecB")
            with (tc.For_i(0, TH // RN) if "crf" not in skips else tc.For_i(0, 1)) as iv:
                nc.vector.tensor_copy(ecA, expF[:, ds(RN * iv, RN)])
                nc.vector.tensor_copy(ecB, expF[:, ds(T - RN - RN * iv, RN)])
                for u in range(RN):
                    # alpha: psA = E @ alpha ; alpha = e_t * psA
                    psA = pss_pool.tile([K, 1], f32, tag="ps0", name=f"psA_{u}")
                    nc.tensor.matmul(psA, EexpT, alpha, start=True, stop=True)
                    # beta: gam = e_{T-1-i} * beta (ACT), then psB = E^T @ gam
                    nc.scalar.activation(
                        gam, beta, AF.Copy, scale=ecB[:, RN - 1 - u : RN - u],
                    )
                    nc.scalar.activation(
                        alpha, psA, AF.Copy, scale=ecA[:, u : u + 1]
                    )
                    psB = pss_pool.tile([K, 1], f32, tag="ps1", name=f"psB_{u}")
                    nc.tensor.matmul(psB, Eexp, gam, start=True, stop=True)
                    nc.vector.tensor_copy(beta, psB)
                # renorm both streams; store norms
                psSA = pss_pool.tile([K, 1], f32, tag="ps0", name="psSA")
                nc.tensor.matmul(psSA, ones48, alpha, start=True, stop=True)
                nc.vector.reciprocal(rsA, psSA)
                nc.vector.tensor_tensor(out=alpha, in0=alpha, in1=rsA, op=ALU.mult)
                nc.vector.tensor_copy(Ssb[:, ds(iv, 1)], psSA[0:1, :])
                psSB = pss_pool.tile([K, 1], f32, tag="ps1", name="psSB")
                nc.tensor.matmul(psSB, ones48, beta, start=True, stop=True)
                nc.vector.reciprocal(rsB, psSB)
                nc.vector.tensor_tensor(out=beta, in0=beta, in1=rsB, op=ALU.mult)
                nc.vector.tensor_copy(Ssb[:, ds(NSA + iv, 1)], psSB[0:1, :])
            # final: dot(alpha_{TH-1}, beta_{TH-1})
            psZ = pss_pool.tile([1, 1], f32, tag="ps0", name="psZ")
            nc.tensor.matmul(psZ, alpha, beta, start=True, stop=True)
            nc.vector.tensor_copy(Ssb[:, NSA + NSB : NSA + NSB + 1], psZ)
            nc.sync.dma_start(out=snorm_out[:, :], in_=Ssb)
    nc.compile()
    return nc


# ----------------------------------------------------------------------------
# entry point
# ----------------------------------------------------------------------------

def _postprocess(r0, inputs):
    feats = r0["feats"].astype(np.float64)  # [K, T]
    sn = r0["snorm"].astype(np.float64)[0]
    Z = np.log(sn).sum()

    tags = np.asarray(inputs["tags"]).astype(np.int64)
    trans = np.asarray(inputs["transitions"]).astype(np.float64)
    ext = np.concatenate([[START_IDX], tags])
    score = trans[ext[1:], ext[:-1]].sum() + feats[tags, np.arange(T)].sum()
    score += trans[END_IDX, tags[-1]]
    return np.float32(Z - score)


def kernel(**inputs) -> np.ndarray:
    if "nc" not in _CACHE:
        _CACHE["nc"] = build()
    nc = _CACHE["nc"]
    in_map = _prep(inputs)
    res = run_bass_kernel_spmd(nc, [in_map], [0])
    return _postprocess(res.results[0], inputs)
